# revision 8
# baseline (speedup 1.0000x reference)
"""Trainium2 Bass kernel for DecoderWithoutAttention (LSTM caption decoder).

Strategy (8 NeuronCores, one chip):
  - Gate-sharded recurrence: core j owns hidden units [j*128,(j+1)*128) across
    all 4 LSTM gates (4 m-tiles of 128 gate rows). State is kept transposed
    (h.T: [hid, batch]) so each step's GEMM gates.T = W @ h.T needs no
    transposes. After each step the fp16 h-shard [128,128] is AllGathered so
    every core has the full h.T [1024,128] for the next step and for fc.
  - Vocab-sharded fc: core j owns vocab rows [j*1250,(j+1)*1250); fc uses the
    h.T k-tiles as stationary operands, fc_W.T slices as moving operand.
  - x-projections (W_ih @ x.T) are folded into each step's PSUM accumulation
    groups (own bank per gate m-tile); they have no AllGather dependency so
    they run during the AG wait window.
  - Image GEMM (x_img.T = img_W @ enc.T) is K-sharded (12544 of 100352 per
    core) + one fp16 AllReduce.
  - All matmuls fp16 inputs, fp32 PSUM accumulation; elementwise in fp32.
  - Inputs are host-pre-arranged into the exact SBUF layouts so every load is
    a contiguous DMA; DMA issue is spread across engine sequencers.
Host side: stable sort by caption length, embedding gather, weight transposes,
final concat of per-core vocab slices.
"""
import os
import sys

if "/opt/trn_rl_repo" not in sys.path:
    sys.path.insert(0, "/opt/trn_rl_repo")

import numpy as np

import concourse.bass as bass
import concourse.bacc as bacc
import concourse.mybir as mybir
from concourse import tile
from concourse.bass_utils import run_bass_kernel_spmd

B = 128
ENC_HW = 14
IMG_DIM = 512
IMG_K = IMG_DIM * ENC_HW * ENC_HW  # 100352
EMB = 512
HID = 1024
VOCAB = 10000
MAX_CAP = 20
T = MAX_CAP - 1  # 19 decode steps
N_CORES = 8
KSH = IMG_K // N_CORES          # 12544 img contraction per core
KT_IMG = KSH // 128             # 98 k-tiles
IMG_CH = 7                      # k-tiles per img weight chunk
N_CH = KT_IMG // IMG_CH         # 14 chunks
VSH = VOCAB // N_CORES          # 1250 vocab rows per core
HSH = HID // N_CORES            # 128 hidden units per core
RG = [list(range(N_CORES))]

F16 = mybir.dt.float16
F32 = mybir.dt.float32
AF = mybir.ActivationFunctionType

FC_N = [512, 512, VSH - 1024]   # fc vocab slices (PSUM bank is 512 fp32)

LAST_RESULT = [None]  # exec-time introspection for test harness


def _emit(nc, tc, io, has_fcb):
    from contextlib import ExitStack
    sync, gps, pe, act, dve = nc.sync, nc.gpsimd, nc.tensor, nc.scalar, nc.vector

    es = ExitStack()
    wp = es.enter_context(tc.tile_pool(name="wp", bufs=1))
    dramp = es.enter_context(tc.tile_pool(name="dramp", bufs=1, space="DRAM"))

    # ---- persistent SBUF ----
    whh = wp.tile([128, 8, 512], F16, name="whh")          # W_hh.T shard
    wih = wp.tile([128, 4, 512], F16, name="wih")          # W_ih.T shard
    fcw = wp.tile([128, 8, VSH], F16, name="fcw")          # fc_W.T shard
    xemb = wp.tile([128, 4, T - 1, 128], F16, name="xemb")  # emb.T, steps 1..18
    ximg = wp.tile([128, 4, 128], F16, name="ximg")        # x_img.T (filled on dev)
    bias = wp.tile([128, 4], F32, name="bias")
    imgb = wp.tile([128, 4], F32, name="imgb")
    mask = wp.tile([128, T], F32, name="mask")
    fcb = wp.tile([1, VSH], F16, name="fcb")
    ones1 = wp.tile([1, 128], F16, name="ones1")
    c_st = wp.tile([128, 128], F32, name="c_st")           # cell state shard

    # weight loads: contiguous, split across queues/engines
    sync.dma_start(whh[:, 0:4, :], io["whh_t"][:, 0:4, :])
    act.dma_start(whh[:, 4:8, :], io["whh_t"][:, 4:8, :])
    sync.dma_start(wih[:], io["wih_t"][:])
    sync.dma_start(fcw[:, 0:4, :], io["fcw_t"][:, 0:4, :])
    act.dma_start(fcw[:, 4:8, :], io["fcw_t"][:, 4:8, :])
    sync.dma_start(xemb[:, 0:2, :, :], io["embt"][:, 0:2, :, :])
    act.dma_start(xemb[:, 2:4, :, :], io["embt"][:, 2:4, :, :])
    sync.dma_start(bias[:], io["bias"][:])
    sync.dma_start(imgb[:], io["imgb"][:])
    sync.dma_start(mask[:], io["mask"][:])
    sync.dma_start(fcb[:], io["fcb"][:])
    dve.memset(ones1[:], 1.0)

    # ---- image GEMM: x_img.T partial = img_W.T_shard.T @ enc.T_shard ----
    with tc.tile_pool(name="imgp", bufs=2) as imgp, \
         tc.tile_pool(name="encp", bufs=1) as encp, \
         tc.tile_pool(name="imgps", bufs=1, space="PSUM") as imgps:
        enct = encp.tile([128, KT_IMG, 128], F16, name="enct")
        for q in range(4):
            lo, hi = q * 25, min((q + 1) * 25, KT_IMG)
            eng = sync if q % 2 == 0 else act
            eng.dma_start(enct[:, lo:hi, :], io["enct"][:, lo:hi, :])
        ps_img = [imgps.tile([128, 128], F32, name=f"ps_img{m}") for m in range(4)]
        for c in range(N_CH):
            wch = imgp.tile([128, IMG_CH, 512], F16, tag="wch", name="wch")
            eng = sync if c % 2 == 0 else act
            eng.dma_start(wch[:, :, 0:256], io["imgwt"][c][:, :, 0:256])
            eng2 = act if c % 2 == 0 else sync
            eng2.dma_start(wch[:, :, 256:512], io["imgwt"][c][:, :, 256:512])
            for k in range(IMG_CH):
                kk = c * IMG_CH + k
                for m in range(4):
                    pe.matmul(ps_img[m][:],
                              wch[:, k, m * 128:(m + 1) * 128],
                              enct[:, kk, :],
                              start=(kk == 0), stop=(kk == KT_IMG - 1))
        ximg_f = imgp.tile([128, 512], F16, name="ximg_f")
        for m in range(4):
            act.copy(ximg_f[:, m * 128:(m + 1) * 128], ps_img[m][:])
        ar_in = dramp.tile([128, 512], F16, name="ar_in")
        ar_out = dramp.tile([128, 512], F16, name="ar_out", addr_space="Shared")
        sync.dma_start(ar_in[:], ximg_f[:])
        gps.collective_compute("AllReduce", mybir.AluOpType.add,
                               replica_groups=RG,
                               ins=[ar_in.opt()], outs=[ar_out.opt()])
        ximg_full = imgp.tile([128, 512], F16, name="ximg_full")
        sync.dma_start(ximg_full[:], ar_out[:])
        for m in range(4):
            dve.tensor_scalar_add(ximg[:, m, :],
                                  ximg_full[:, m * 128:(m + 1) * 128],
                                  imgb[:, m:m + 1])

    # ---- recurrence ----
    with tc.tile_pool(name="psg", bufs=1, space="PSUM") as psg, \
         tc.tile_pool(name="psf", bufs=1, space="PSUM") as psf, \
         tc.tile_pool(name="dyn", bufs=2) as dyn, \
         tc.tile_pool(name="agp", bufs=2, space="DRAM") as agp:

        hk_prev = None  # 8 SBUF tiles [128,128] f16: full h.T of prev step

        def emit_fc_mm(hk):
            """fc matmuls for a step; returns psum tiles."""
            tiles = []
            off = 0
            for n, nw in enumerate(FC_N):
                psn = psf.tile([128, 512], F32, tag=f"psf{n}", name=f"psf{n}")
                if has_fcb:
                    pe.matmul(psn[:, :nw], ones1[:], fcb[:, off:off + nw],
                              start=True, stop=False)
                for k in range(8):
                    pe.matmul(psn[:, :nw], hk[k][:],
                              fcw[:, k, off:off + nw],
                              start=(k == 0 and not has_fcb), stop=(k == 7))
                tiles.append(psn)
                off += nw
            return tiles

        def emit_fc_out(s, tiles):
            """mask + store preds for step s (emitted off the critical path)."""
            off = 0
            for n, nw in enumerate(FC_N):
                pr = dyn.tile([128, 512], F32, tag=f"pr{n}", name=f"pr{n}")
                dve.tensor_scalar_mul(pr[:, :nw], tiles[n][:, :nw],
                                      mask[:, s:s + 1])
                gps.dma_start(io["preds"][:, s, off:off + nw], pr[:, :nw])
                off += nw

        fc_tiles = None
        for t in range(T):
            # gates.T accumulation, one PSUM bank per gate m-tile.
            # x-side first (no AG dependency -> fills the AG wait window),
            # then the h-side after the AllGather of the previous step.
            ps_g = [psg.tile([128, 128], F32, tag=f"psg{m}", name=f"psg{m}")
                    for m in range(4)]
            for m in range(4):
                for k in range(4):
                    xsl = ximg[:, k, :] if t == 0 else xemb[:, k, t - 1, :]
                    pe.matmul(ps_g[m][:],
                              wih[:, k, m * 128:(m + 1) * 128], xsl,
                              start=(k == 0), stop=(t == 0 and k == 3))
            if t > 0:
                # k-outer: each k-block only needs hk[k], so the PE chases
                # the 8 h-tile DMA arrivals instead of waiting for the last
                for k in range(8):
                    for m in range(4):
                        pe.matmul(ps_g[m][:],
                                  whh[:, k, m * 128:(m + 1) * 128],
                                  hk_prev[k][:],
                                  start=False, stop=(k == 7))
                # fc of previous step rides the PE queue after this step's
                # gates (same AG dependency, off the critical chain)
                fc_tiles = emit_fc_mm(hk_prev)

            # activations in dependency-friendly order: i, g first (-> i*g),
            # then f (-> f*c), then o
            g_act = [None] * 4
            for m in (0, 2, 1, 3):
                a = dyn.tile([128, 128], F32, tag=f"ga{m}", name=f"ga{m}")
                act.activation(a[:], ps_g[m][:],
                               AF.Tanh if m == 2 else AF.Sigmoid,
                               bias=bias[:, m:m + 1])
                g_act[m] = a
            i_g, f_g, g_g, o_g = g_act
            if t == 0:
                dve.tensor_mul(c_st[:], i_g[:], g_g[:])
            else:
                t2 = dyn.tile([128, 128], F32, tag="t2", name="t2")
                t1 = dyn.tile([128, 128], F32, tag="t1", name="t1")
                dve.tensor_mul(t2[:], i_g[:], g_g[:])
                dve.tensor_mul(t1[:], f_g[:], c_st[:])
                dve.tensor_add(c_st[:], t1[:], t2[:])
            tc_ = dyn.tile([128, 128], F32, tag="tc", name="tc_")
            act.activation(tc_[:], c_st[:], AF.Tanh)
            h16 = dyn.tile([128, 128], F16, tag="h16", name="h16")
            dve.tensor_mul(h16[:], o_g[:], tc_[:])

            # exchange: shard -> full h.T
            bnc = agp.tile([128, 128], F16, tag="bnc", name="bnc")
            gat = agp.tile([HID, 128], F16, tag="gat", name="gat",
                           addr_space="Shared")
            sync.dma_start(bnc[:], h16[:])
            gps.collective_compute("AllGather", mybir.AluOpType.bypass,
                                   replica_groups=RG,
                                   ins=[bnc.opt()], outs=[gat.opt()])
            # preds of the previous step: after the critical-path ops
            if t > 0:
                emit_fc_out(t - 1, fc_tiles)
            hk = []
            for k in range(8):
                ht = dyn.tile([128, 128], F16, tag=f"hk{k}", name=f"hk{k}")
                eng = sync if k % 2 == 0 else act
                eng.dma_start(ht[:], gat[k * 128:(k + 1) * 128, :])
                hk.append(ht)
            hk_prev = hk

        fc_tiles = emit_fc_mm(hk_prev)
        emit_fc_out(T - 1, fc_tiles)

    es.close()


def _build(has_fcb):
    nc = bacc.Bacc("TRN2", target_bir_lowering=False, debug=False,
                   num_devices=N_CORES)
    io = {
        "whh_t": nc.dram_tensor("whh_t", [128, 8, 512], F16, kind="ExternalInput").ap(),
        "wih_t": nc.dram_tensor("wih_t", [128, 4, 512], F16, kind="ExternalInput").ap(),
        "fcw_t": nc.dram_tensor("fcw_t", [128, 8, VSH], F16, kind="ExternalInput").ap(),
        "fcb": nc.dram_tensor("fcb", [1, VSH], F16, kind="ExternalInput").ap(),
        "bias": nc.dram_tensor("bias", [128, 4], F32, kind="ExternalInput").ap(),
        "imgb": nc.dram_tensor("imgb", [128, 4], F32, kind="ExternalInput").ap(),
        "mask": nc.dram_tensor("mask", [128, T], F32, kind="ExternalInput").ap(),
        "embt": nc.dram_tensor("embt", [128, 4, T - 1, 128], F16, kind="ExternalInput").ap(),
        "enct": nc.dram_tensor("enct", [128, KT_IMG, 128], F16, kind="ExternalInput").ap(),
        "imgwt": nc.dram_tensor("imgwt", [N_CH, 128, IMG_CH, 512], F16, kind="ExternalInput").ap(),
        "preds": nc.dram_tensor("preds", [128, T, VSH], F32, kind="ExternalOutput").ap(),
    }
    with tile.TileContext(nc) as tc:
        _emit(nc, tc, io, has_fcb)
    nc.compile()
    return nc


_PROGRAM = {}


def _program(has_fcb):
    if has_fcb not in _PROGRAM:
        _PROGRAM[has_fcb] = _build(has_fcb)
    return _PROGRAM[has_fcb]


def _host_prep(inputs):
    enc = np.asarray(inputs["encoder_out"], dtype=np.float32)
    caps = np.asarray(inputs["encoded_captions"])
    clen = np.asarray(inputs["caption_lengths"])
    emb_W = np.asarray(inputs["emb_W"], dtype=np.float32)
    img_W = np.asarray(inputs["img_W"], dtype=np.float32)
    img_b = np.asarray(inputs["img_b"], dtype=np.float32)
    W_ih = np.asarray(inputs["W_ih"], dtype=np.float32)
    W_hh = np.asarray(inputs["W_hh"], dtype=np.float32)
    b_ih = np.asarray(inputs["b_ih"], dtype=np.float32)
    b_hh = np.asarray(inputs["b_hh"], dtype=np.float32)
    fc_W = np.asarray(inputs["fc_W"], dtype=np.float32)
    fc_b = np.asarray(inputs["fc_b"], dtype=np.float32)

    lens = clen[:, 0]
    sort_ind = np.argsort(-lens, kind="stable")
    lens_s = lens[sort_ind]
    dec_len = lens_s - 1
    caps_s = caps[sort_ind]

    encf = enc.reshape(B, -1)[sort_ind]                      # [B, 100352]
    enct = np.ascontiguousarray(encf.T).astype(np.float16)   # [100352, B]
    imgwt = np.ascontiguousarray(img_W.T).astype(np.float16)  # [100352, 512]

    emb_x = emb_W[caps_s[:, :T - 1]]                         # [B, 18, EMB]
    # [EMB, 18, B] -> [128, 4, 18, 128] (partition-major SBUF layout)
    embt = np.ascontiguousarray(
        emb_x.transpose(2, 1, 0).reshape(4, 128, T - 1, 128)
        .transpose(1, 0, 2, 3))
    embt = embt.astype(np.float16)

    bsum = b_ih + b_hh
    maskf = (dec_len[:, None] >= np.arange(T)[None, :]).astype(np.float32)
    imgb_t = np.ascontiguousarray(img_b.reshape(4, 128).T).astype(np.float32)
    W_ihT = W_ih.T  # [EMB, 4H]
    W_hhT = W_hh.T  # [HID, 4H]
    fc_WT = fc_W.T  # [HID, VOCAB]

    def sbuf_km(a, kt):  # [kt*128, M] -> [128, kt, M]
        return np.ascontiguousarray(
            a.reshape(kt, 128, a.shape[1]).transpose(1, 0, 2))

    in_maps = []
    for j in range(N_CORES):
        rows = np.concatenate(
            [np.arange(g * HID + j * HSH, g * HID + (j + 1) * HSH)
             for g in range(4)])
        vsl = slice(j * VSH, (j + 1) * VSH)
        ksl = slice(j * KSH, (j + 1) * KSH)
        imgw_j = imgwt[ksl].reshape(N_CH, IMG_CH, 128, 512).transpose(0, 2, 1, 3)
        in_maps.append({
            "whh_t": sbuf_km(W_hhT[:, rows].astype(np.float16), 8),
            "wih_t": sbuf_km(W_ihT[:, rows].astype(np.float16), 4),
            "fcw_t": sbuf_km(fc_WT[:, vsl].astype(np.float16), 8),
            "fcb": np.ascontiguousarray(fc_b[vsl])[None, :].astype(np.float16),
            "bias": np.ascontiguousarray(
                bsum[rows].reshape(4, HSH).T).astype(np.float32),
            "imgb": imgb_t,
            "mask": maskf,
            "embt": embt,
            "enct": sbuf_km(enct[ksl], KT_IMG),
            "imgwt": np.ascontiguousarray(imgw_j),
        })
    meta = dict(sort_ind=sort_ind, dec_len=dec_len, caps_s=caps_s,
                has_fcb=bool(np.any(fc_b)))
    return in_maps, meta


def _install_trace_shim():
    """Make run_bass_kernel_spmd(trace=True) work under axon on this image."""
    import types
    try:
        import antenv  # noqa
        if "antenv.axon_hooks" not in sys.modules:
            mod = types.ModuleType("antenv.axon_hooks")
            _hook = [None]
            mod.set_axon_ntff_profile_hook = lambda h: _hook.__setitem__(0, h)
            mod.get_axon_ntff_profile_hook = lambda: _hook[0]
            sys.modules["antenv.axon_hooks"] = mod
            antenv.axon_hooks = mod
        from trn_agent_boot.trn_boot import _ntff_profile_via_ctypes
        sys.modules["antenv.axon_hooks"].set_axon_ntff_profile_hook(
            _ntff_profile_via_ctypes("/opt/axon/libaxon_pjrt.so"))
        import concourse.bass_utils as bu
        bu.upload_artifacts = lambda tmpdir: f"local:{tmpdir}"
    except Exception:
        pass


def kernel(**inputs):
    trace = bool(os.environ.get("BASS_TRACE"))
    if trace:
        _install_trace_shim()
    in_maps, meta = _host_prep(inputs)
    nc = _program(meta["has_fcb"])
    res = run_bass_kernel_spmd(nc, in_maps, list(range(N_CORES)), trace=trace)
    LAST_RESULT[0] = res
    preds = np.concatenate(
        [res.results[c]["preds"] for c in range(N_CORES)], axis=2)
    caps_out = meta["caps_s"]
    dec_out = meta["dec_len"]
    sort_out = meta["sort_ind"].astype(np.int32)
    return preds, caps_out, dec_out, sort_out


# revision 9
# speedup vs baseline: 1.0482x; 1.0482x over previous
"""Trainium2 Bass kernel for DecoderWithoutAttention (LSTM caption decoder).

Strategy (8 NeuronCores, one chip):
  - Gate-sharded recurrence: core j owns hidden units [j*128,(j+1)*128) across
    all 4 LSTM gates (4 m-tiles of 128 gate rows). State is kept transposed
    (h.T: [hid, batch]) so each step's GEMM gates.T = W @ h.T needs no
    transposes. After each step the fp16 h-shard [128,128] is AllGathered so
    every core has the full h.T [1024,128] for the next step and for fc.
  - Vocab-sharded fc: core j owns vocab rows [j*1250,(j+1)*1250); fc uses the
    h.T k-tiles as stationary operands, fc_W.T slices as moving operand.
  - x-projections (W_ih @ x.T) are folded into each step's PSUM accumulation
    groups (own bank per gate m-tile); they have no AllGather dependency so
    they run during the AG wait window.
  - Image GEMM (x_img.T = img_W @ enc.T) is K-sharded (12544 of 100352 per
    core) + one fp16 AllReduce.
  - All matmuls fp16 inputs, fp32 PSUM accumulation; elementwise in fp32.
  - Inputs are host-pre-arranged into the exact SBUF layouts so every load is
    a contiguous DMA; DMA issue is spread across engine sequencers.
Host side: stable sort by caption length, embedding gather, weight transposes,
final concat of per-core vocab slices.
"""
import os
import sys

if "/opt/trn_rl_repo" not in sys.path:
    sys.path.insert(0, "/opt/trn_rl_repo")

import numpy as np

import concourse.bass as bass
import concourse.bacc as bacc
import concourse.mybir as mybir
from concourse import tile
from concourse.bass_utils import run_bass_kernel_spmd

B = 128
ENC_HW = 14
IMG_DIM = 512
IMG_K = IMG_DIM * ENC_HW * ENC_HW  # 100352
EMB = 512
HID = 1024
VOCAB = 10000
MAX_CAP = 20
T = MAX_CAP - 1  # 19 decode steps
N_CORES = 8
KSH = IMG_K // N_CORES          # 12544 img contraction per core
KT_IMG = KSH // 128             # 98 k-tiles
IMG_CH = 7                      # k-tiles per img weight chunk
N_CH = KT_IMG // IMG_CH         # 14 chunks
VSH = VOCAB // N_CORES          # 1250 vocab rows per core
HSH = HID // N_CORES            # 128 hidden units per core
RG = [list(range(N_CORES))]

F16 = mybir.dt.float16
F32 = mybir.dt.float32
AF = mybir.ActivationFunctionType

FC_N = [512, 512, VSH - 1024]   # fc vocab slices (PSUM bank is 512 fp32)

LAST_RESULT = [None]  # exec-time introspection for test harness


def _emit(nc, tc, io, has_fcb):
    from contextlib import ExitStack
    sync, gps, pe, act, dve = nc.sync, nc.gpsimd, nc.tensor, nc.scalar, nc.vector

    es = ExitStack()
    wp = es.enter_context(tc.tile_pool(name="wp", bufs=1))
    dramp = es.enter_context(tc.tile_pool(name="dramp", bufs=1, space="DRAM"))

    # ---- persistent SBUF ----
    whh = wp.tile([128, 8, 512], F16, name="whh")          # W_hh.T shard
    wih = wp.tile([128, 4, 512], F16, name="wih")          # W_ih.T shard
    fcw = wp.tile([128, 8, VSH], F16, name="fcw")          # fc_W.T shard
    xemb = wp.tile([128, 4, T - 1, 128], F16, name="xemb")  # emb.T, steps 1..18
    ximg = wp.tile([128, 4, 128], F16, name="ximg")        # x_img.T (filled on dev)
    bias = wp.tile([128, 4], F32, name="bias")
    imgb = wp.tile([128, 4], F32, name="imgb")
    mask = wp.tile([128, T], F32, name="mask")
    fcb = wp.tile([1, VSH], F16, name="fcb")
    ones1 = wp.tile([1, 128], F16, name="ones1")
    c_st = wp.tile([128, 128], F32, name="c_st")           # cell state shard

    # small/urgent loads first (wih is needed for step-0 x-side right after
    # the AllReduce); the big recurrence weights are loaded during the AR wait
    sync.dma_start(wih[:], io["wih_t"][:])
    sync.dma_start(bias[:], io["bias"][:])
    sync.dma_start(imgb[:], io["imgb"][:])
    sync.dma_start(mask[:], io["mask"][:])
    sync.dma_start(fcb[:], io["fcb"][:])
    dve.memset(ones1[:], 1.0)

    # ---- image GEMM: x_img.T partial = img_W.T_shard.T @ enc.T_shard ----
    with tc.tile_pool(name="imgp", bufs=2) as imgp, \
         tc.tile_pool(name="encp", bufs=1) as encp, \
         tc.tile_pool(name="imgps", bufs=1, space="PSUM") as imgps:
        enct = encp.tile([128, KT_IMG, 128], F16, name="enct")
        for q in range(4):
            lo, hi = q * 25, min((q + 1) * 25, KT_IMG)
            eng = sync if q % 2 == 0 else act
            eng.dma_start(enct[:, lo:hi, :], io["enct"][:, lo:hi, :])
        ps_img = [imgps.tile([128, 128], F32, name=f"ps_img{m}") for m in range(4)]
        for c in range(N_CH):
            wch = imgp.tile([128, IMG_CH, 512], F16, tag="wch", name="wch")
            eng = sync if c % 2 == 0 else act
            eng.dma_start(wch[:, :, 0:256], io["imgwt"][c][:, :, 0:256])
            eng2 = act if c % 2 == 0 else sync
            eng2.dma_start(wch[:, :, 256:512], io["imgwt"][c][:, :, 256:512])
            for k in range(IMG_CH):
                kk = c * IMG_CH + k
                for m in range(4):
                    pe.matmul(ps_img[m][:],
                              wch[:, k, m * 128:(m + 1) * 128],
                              enct[:, kk, :],
                              start=(kk == 0), stop=(kk == KT_IMG - 1))
        ximg_f = imgp.tile([128, 512], F16, name="ximg_f")
        for m in range(4):
            act.copy(ximg_f[:, m * 128:(m + 1) * 128], ps_img[m][:])
        ar_in = dramp.tile([128, 512], F16, name="ar_in")
        ar_out = dramp.tile([128, 512], F16, name="ar_out", addr_space="Shared")
        sync.dma_start(ar_in[:], ximg_f[:])
        gps.collective_compute("AllReduce", mybir.AluOpType.add,
                               replica_groups=RG,
                               ins=[ar_in.opt()], outs=[ar_out.opt()])
        # big recurrence weights stream in during the AllReduce wait
        sync.dma_start(whh[:, 0:4, :], io["whh_t"][:, 0:4, :])
        act.dma_start(whh[:, 4:8, :], io["whh_t"][:, 4:8, :])
        sync.dma_start(fcw[:, 0:4, :], io["fcw_t"][:, 0:4, :])
        act.dma_start(fcw[:, 4:8, :], io["fcw_t"][:, 4:8, :])
        sync.dma_start(xemb[:, 0:2, :, :], io["embt"][:, 0:2, :, :])
        act.dma_start(xemb[:, 2:4, :, :], io["embt"][:, 2:4, :, :])
        ximg_full = imgp.tile([128, 512], F16, name="ximg_full")
        sync.dma_start(ximg_full[:], ar_out[:])
        for m in range(4):
            dve.tensor_scalar_add(ximg[:, m, :],
                                  ximg_full[:, m * 128:(m + 1) * 128],
                                  imgb[:, m:m + 1])

    # ---- recurrence ----
    with tc.tile_pool(name="psg", bufs=1, space="PSUM") as psg, \
         tc.tile_pool(name="psf", bufs=1, space="PSUM") as psf, \
         tc.tile_pool(name="dyn", bufs=2) as dyn, \
         tc.tile_pool(name="agp", bufs=2, space="DRAM") as agp:

        hk_prev = None  # 8 SBUF tiles [128,128] f16: full h.T of prev step

        def emit_fc_mm(hk):
            """fc matmuls for a step; returns psum tiles."""
            tiles = []
            off = 0
            for n, nw in enumerate(FC_N):
                psn = psf.tile([128, 512], F32, tag=f"psf{n}", name=f"psf{n}")
                if has_fcb:
                    pe.matmul(psn[:, :nw], ones1[:], fcb[:, off:off + nw],
                              start=True, stop=False)
                for k in range(8):
                    pe.matmul(psn[:, :nw], hk[k],
                              fcw[:, k, off:off + nw],
                              start=(k == 0 and not has_fcb), stop=(k == 7))
                tiles.append(psn)
                off += nw
            return tiles

        def emit_fc_out(s, tiles):
            """mask + store preds for step s (emitted off the critical path)."""
            off = 0
            for n, nw in enumerate(FC_N):
                pr = dyn.tile([128, 512], F32, tag=f"pr{n}", name=f"pr{n}")
                dve.tensor_scalar_mul(pr[:, :nw], tiles[n][:, :nw],
                                      mask[:, s:s + 1])
                gps.dma_start(io["preds"][:, s, off:off + nw], pr[:, :nw])
                off += nw

        fc_tiles = None
        for t in range(T):
            # gates.T accumulation, one PSUM bank per gate m-tile.
            # x-side first (no AG dependency -> fills the AG wait window),
            # then the h-side after the AllGather of the previous step.
            ps_g = [psg.tile([128, 128], F32, tag=f"psg{m}", name=f"psg{m}")
                    for m in range(4)]
            for m in range(4):
                for k in range(4):
                    xsl = ximg[:, k, :] if t == 0 else xemb[:, k, t - 1, :]
                    pe.matmul(ps_g[m][:],
                              wih[:, k, m * 128:(m + 1) * 128], xsl,
                              start=(k == 0), stop=(t == 0 and k == 3))
            if t > 0:
                # k-outer: each k-block only needs hk[k], so the PE chases
                # the 8 h-tile DMA arrivals instead of waiting for the last
                for k in range(8):
                    for m in range(4):
                        pe.matmul(ps_g[m][:],
                                  whh[:, k, m * 128:(m + 1) * 128],
                                  hk_prev[k],
                                  start=False, stop=(k == 7))
                # fc of previous step rides the PE queue after this step's
                # gates (same AG dependency, off the critical chain)
                fc_tiles = emit_fc_mm(hk_prev)

            # activations in dependency-friendly order: i, g first (-> i*g),
            # then f (-> f*c), then o
            g_act = [None] * 4
            for m in (0, 2, 1, 3):
                a = dyn.tile([128, 128], F32, tag=f"ga{m}", name=f"ga{m}")
                act.activation(a[:], ps_g[m][:],
                               AF.Tanh if m == 2 else AF.Sigmoid,
                               bias=bias[:, m:m + 1])
                g_act[m] = a
            i_g, f_g, g_g, o_g = g_act
            if t == 0:
                dve.tensor_mul(c_st[:], i_g[:], g_g[:])
            else:
                t2 = dyn.tile([128, 128], F32, tag="t2", name="t2")
                t1 = dyn.tile([128, 128], F32, tag="t1", name="t1")
                dve.tensor_mul(t2[:], i_g[:], g_g[:])
                dve.tensor_mul(t1[:], f_g[:], c_st[:])
                dve.tensor_add(c_st[:], t1[:], t2[:])
            tc_ = dyn.tile([128, 128], F32, tag="tc", name="tc_")
            act.activation(tc_[:], c_st[:], AF.Tanh)
            h16 = dyn.tile([128, 128], F16, tag="h16", name="h16")
            dve.tensor_mul(h16[:], o_g[:], tc_[:])

            # exchange: shard -> full h.T
            bnc = agp.tile([128, 128], F16, tag="bnc", name="bnc")
            gat = agp.tile([HID, 128], F16, tag="gat", name="gat",
                           addr_space="Shared")
            act.dma_start(bnc[:], h16[:])
            gps.collective_compute("AllGather", mybir.AluOpType.bypass,
                                   replica_groups=RG,
                                   ins=[bnc.opt()], outs=[gat.opt()])
            # preds of the previous step: after the critical-path ops
            if t > 0:
                emit_fc_out(t - 1, fc_tiles)
            hk = []
            for p in range(4):
                ht = dyn.tile([128, 2, 128], F16, tag=f"hkp{p}", name=f"hkp{p}")
                eng = sync if p % 2 == 0 else act
                eng.dma_start(ht[:], gat.rearrange("(k p) b -> p k b", p=128)
                              [:, 2 * p:2 * p + 2, :])
                hk.append(ht)
            hk_prev = [hk[k // 2][:, k % 2, :] for k in range(8)]

        fc_tiles = emit_fc_mm(hk_prev)
        emit_fc_out(T - 1, fc_tiles)

    es.close()


def _build(has_fcb):
    nc = bacc.Bacc("TRN2", target_bir_lowering=False, debug=False,
                   num_devices=N_CORES)
    io = {
        "whh_t": nc.dram_tensor("whh_t", [128, 8, 512], F16, kind="ExternalInput").ap(),
        "wih_t": nc.dram_tensor("wih_t", [128, 4, 512], F16, kind="ExternalInput").ap(),
        "fcw_t": nc.dram_tensor("fcw_t", [128, 8, VSH], F16, kind="ExternalInput").ap(),
        "fcb": nc.dram_tensor("fcb", [1, VSH], F16, kind="ExternalInput").ap(),
        "bias": nc.dram_tensor("bias", [128, 4], F32, kind="ExternalInput").ap(),
        "imgb": nc.dram_tensor("imgb", [128, 4], F32, kind="ExternalInput").ap(),
        "mask": nc.dram_tensor("mask", [128, T], F32, kind="ExternalInput").ap(),
        "embt": nc.dram_tensor("embt", [128, 4, T - 1, 128], F16, kind="ExternalInput").ap(),
        "enct": nc.dram_tensor("enct", [128, KT_IMG, 128], F16, kind="ExternalInput").ap(),
        "imgwt": nc.dram_tensor("imgwt", [N_CH, 128, IMG_CH, 512], F16, kind="ExternalInput").ap(),
        "preds": nc.dram_tensor("preds", [128, T, VSH], F32, kind="ExternalOutput").ap(),
    }
    with tile.TileContext(nc) as tc:
        _emit(nc, tc, io, has_fcb)
    nc.compile()
    return nc


_PROGRAM = {}


def _program(has_fcb):
    if has_fcb not in _PROGRAM:
        _PROGRAM[has_fcb] = _build(has_fcb)
    return _PROGRAM[has_fcb]


def _host_prep(inputs):
    enc = np.asarray(inputs["encoder_out"], dtype=np.float32)
    caps = np.asarray(inputs["encoded_captions"])
    clen = np.asarray(inputs["caption_lengths"])
    emb_W = np.asarray(inputs["emb_W"], dtype=np.float32)
    img_W = np.asarray(inputs["img_W"], dtype=np.float32)
    img_b = np.asarray(inputs["img_b"], dtype=np.float32)
    W_ih = np.asarray(inputs["W_ih"], dtype=np.float32)
    W_hh = np.asarray(inputs["W_hh"], dtype=np.float32)
    b_ih = np.asarray(inputs["b_ih"], dtype=np.float32)
    b_hh = np.asarray(inputs["b_hh"], dtype=np.float32)
    fc_W = np.asarray(inputs["fc_W"], dtype=np.float32)
    fc_b = np.asarray(inputs["fc_b"], dtype=np.float32)

    lens = clen[:, 0]
    sort_ind = np.argsort(-lens, kind="stable")
    lens_s = lens[sort_ind]
    dec_len = lens_s - 1
    caps_s = caps[sort_ind]

    encf = enc.reshape(B, -1)[sort_ind]                      # [B, 100352]
    enct = np.ascontiguousarray(encf.T).astype(np.float16)   # [100352, B]
    imgwt = np.ascontiguousarray(img_W.T).astype(np.float16)  # [100352, 512]

    emb_x = emb_W[caps_s[:, :T - 1]]                         # [B, 18, EMB]
    # [EMB, 18, B] -> [128, 4, 18, 128] (partition-major SBUF layout)
    embt = np.ascontiguousarray(
        emb_x.transpose(2, 1, 0).reshape(4, 128, T - 1, 128)
        .transpose(1, 0, 2, 3))
    embt = embt.astype(np.float16)

    bsum = b_ih + b_hh
    maskf = (dec_len[:, None] >= np.arange(T)[None, :]).astype(np.float32)
    imgb_t = np.ascontiguousarray(img_b.reshape(4, 128).T).astype(np.float32)
    W_ihT = W_ih.T  # [EMB, 4H]
    W_hhT = W_hh.T  # [HID, 4H]
    fc_WT = fc_W.T  # [HID, VOCAB]

    def sbuf_km(a, kt):  # [kt*128, M] -> [128, kt, M]
        return np.ascontiguousarray(
            a.reshape(kt, 128, a.shape[1]).transpose(1, 0, 2))

    in_maps = []
    for j in range(N_CORES):
        rows = np.concatenate(
            [np.arange(g * HID + j * HSH, g * HID + (j + 1) * HSH)
             for g in range(4)])
        vsl = slice(j * VSH, (j + 1) * VSH)
        ksl = slice(j * KSH, (j + 1) * KSH)
        imgw_j = imgwt[ksl].reshape(N_CH, IMG_CH, 128, 512).transpose(0, 2, 1, 3)
        in_maps.append({
            "whh_t": sbuf_km(W_hhT[:, rows].astype(np.float16), 8),
            "wih_t": sbuf_km(W_ihT[:, rows].astype(np.float16), 4),
            "fcw_t": sbuf_km(fc_WT[:, vsl].astype(np.float16), 8),
            "fcb": np.ascontiguousarray(fc_b[vsl])[None, :].astype(np.float16),
            "bias": np.ascontiguousarray(
                bsum[rows].reshape(4, HSH).T).astype(np.float32),
            "imgb": imgb_t,
            "mask": maskf,
            "embt": embt,
            "enct": sbuf_km(enct[ksl], KT_IMG),
            "imgwt": np.ascontiguousarray(imgw_j),
        })
    meta = dict(sort_ind=sort_ind, dec_len=dec_len, caps_s=caps_s,
                has_fcb=bool(np.any(fc_b)))
    return in_maps, meta


def _install_trace_shim():
    """Make run_bass_kernel_spmd(trace=True) work under axon on this image."""
    import types
    try:
        import antenv  # noqa
        if "antenv.axon_hooks" not in sys.modules:
            mod = types.ModuleType("antenv.axon_hooks")
            _hook = [None]
            mod.set_axon_ntff_profile_hook = lambda h: _hook.__setitem__(0, h)
            mod.get_axon_ntff_profile_hook = lambda: _hook[0]
            sys.modules["antenv.axon_hooks"] = mod
            antenv.axon_hooks = mod
        from trn_agent_boot.trn_boot import _ntff_profile_via_ctypes
        sys.modules["antenv.axon_hooks"].set_axon_ntff_profile_hook(
            _ntff_profile_via_ctypes("/opt/axon/libaxon_pjrt.so"))
        import concourse.bass_utils as bu
        bu.upload_artifacts = lambda tmpdir: f"local:{tmpdir}"
    except Exception:
        pass


def kernel(**inputs):
    trace = bool(os.environ.get("BASS_TRACE"))
    if trace:
        _install_trace_shim()
    in_maps, meta = _host_prep(inputs)
    nc = _program(meta["has_fcb"])
    res = run_bass_kernel_spmd(nc, in_maps, list(range(N_CORES)), trace=trace)
    LAST_RESULT[0] = res
    preds = np.concatenate(
        [res.results[c]["preds"] for c in range(N_CORES)], axis=2)
    caps_out = meta["caps_s"]
    dec_out = meta["dec_len"]
    sort_out = meta["sort_ind"].astype(np.int32)
    return preds, caps_out, dec_out, sort_out


# revision 10
# speedup vs baseline: 1.0678x; 1.0187x over previous
"""Trainium2 Bass kernel for DecoderWithoutAttention (LSTM caption decoder).

Strategy (8 NeuronCores, one chip):
  - Gate-sharded recurrence: core j owns hidden units [j*128,(j+1)*128) across
    all 4 LSTM gates (4 m-tiles of 128 gate rows). State is kept transposed
    (h.T: [hid, batch]) so each step's GEMM gates.T = W @ h.T needs no
    transposes. After each step the fp16 h-shard [128,128] is AllGathered so
    every core has the full h.T [1024,128] for the next step and for fc.
  - Vocab-sharded fc: core j owns vocab rows [j*1250,(j+1)*1250); fc uses the
    h.T k-tiles as stationary operands, fc_W.T slices as moving operand.
  - x-projections (W_ih @ x.T) are folded into each step's PSUM accumulation
    groups (own bank per gate m-tile); they have no AllGather dependency so
    they run during the AG wait window.
  - Image GEMM (x_img.T = img_W @ enc.T) is K-sharded (12544 of 100352 per
    core) + one fp16 AllReduce.
  - All matmuls fp16 inputs, fp32 PSUM accumulation; elementwise in fp32.
  - Inputs are host-pre-arranged into the exact SBUF layouts so every load is
    a contiguous DMA; DMA issue is spread across engine sequencers.
Host side: stable sort by caption length, embedding gather, weight transposes,
final concat of per-core vocab slices.
"""
import os
import sys

if "/opt/trn_rl_repo" not in sys.path:
    sys.path.insert(0, "/opt/trn_rl_repo")

import numpy as np

import concourse.bass as bass
import concourse.bacc as bacc
import concourse.mybir as mybir
from concourse import tile
from concourse.bass_utils import run_bass_kernel_spmd

B = 128
ENC_HW = 14
IMG_DIM = 512
IMG_K = IMG_DIM * ENC_HW * ENC_HW  # 100352
EMB = 512
HID = 1024
VOCAB = 10000
MAX_CAP = 20
T = MAX_CAP - 1  # 19 decode steps
N_CORES = 8
KSH = IMG_K // N_CORES          # 12544 img contraction per core
KT_IMG = KSH // 128             # 98 k-tiles
IMG_CH = 7                      # k-tiles per img weight chunk
N_CH = KT_IMG // IMG_CH         # 14 chunks
VSH = VOCAB // N_CORES          # 1250 vocab rows per core
HSH = HID // N_CORES            # 128 hidden units per core
RG = [list(range(N_CORES))]

F16 = mybir.dt.float16
F32 = mybir.dt.float32
AF = mybir.ActivationFunctionType

FC_N = [512, 512, VSH - 1024]   # fc vocab slices (PSUM bank is 512 fp32)

LAST_RESULT = [None]  # exec-time introspection for test harness


def _emit(nc, tc, io, has_fcb):
    from contextlib import ExitStack
    sync, gps, pe, act, dve = nc.sync, nc.gpsimd, nc.tensor, nc.scalar, nc.vector

    es = ExitStack()
    wp = es.enter_context(tc.tile_pool(name="wp", bufs=1))
    dramp = es.enter_context(tc.tile_pool(name="dramp", bufs=1, space="DRAM"))

    # ---- persistent SBUF ----
    whh = wp.tile([128, 8, 512], F16, name="whh")          # W_hh.T shard
    wih = wp.tile([128, 4, 512], F16, name="wih")          # W_ih.T shard
    fcw = wp.tile([128, 8, VSH], F16, name="fcw")          # fc_W.T shard
    xemb = wp.tile([128, 4, T - 1, 128], F16, name="xemb")  # emb.T, steps 1..18
    ximg = wp.tile([128, 4, 128], F16, name="ximg")        # x_img.T (filled on dev)
    bias = wp.tile([128, 4], F32, name="bias")
    imgb = wp.tile([128, 4], F32, name="imgb")
    mask = wp.tile([128, T], F32, name="mask")
    fcb = wp.tile([1, VSH], F16, name="fcb")
    ones1 = wp.tile([1, 128], F16, name="ones1")
    c_st = wp.tile([128, 128], F32, name="c_st")           # cell state shard

    # small/urgent loads first (wih is needed for step-0 x-side right after
    # the AllReduce); the big recurrence weights are loaded during the AR wait
    sync.dma_start(wih[:], io["wih_t"][:])
    sync.dma_start(bias[:], io["bias"][:])
    sync.dma_start(imgb[:], io["imgb"][:])
    sync.dma_start(mask[:], io["mask"][:])
    sync.dma_start(fcb[:], io["fcb"][:])
    dve.memset(ones1[:], 1.0)

    # tiny AllGather barrier: absorbs cross-core launch skew while the
    # weight/img DMAs stream, so the later collectives see aligned ranks
    bar_in = dramp.tile([1, 4], F32, name="bar_in")
    bar_out = dramp.tile([N_CORES, 4], F32, name="bar_out", addr_space="Shared")
    sync.dma_start(bar_in[:], io["bias"][0:1, 0:4])
    gps.collective_compute("AllGather", mybir.AluOpType.bypass,
                           replica_groups=RG,
                           ins=[bar_in.opt()], outs=[bar_out.opt()])

    # ---- image GEMM: x_img.T partial = img_W.T_shard.T @ enc.T_shard ----
    with tc.tile_pool(name="imgp", bufs=2) as imgp, \
         tc.tile_pool(name="encp", bufs=1) as encp, \
         tc.tile_pool(name="imgps", bufs=1, space="PSUM") as imgps:
        enct = encp.tile([128, KT_IMG, 128], F16, name="enct")
        for q in range(4):
            lo, hi = q * 25, min((q + 1) * 25, KT_IMG)
            eng = sync if q % 2 == 0 else act
            eng.dma_start(enct[:, lo:hi, :], io["enct"][:, lo:hi, :])
        ps_img = [imgps.tile([128, 128], F32, name=f"ps_img{m}") for m in range(4)]
        for c in range(N_CH):
            wch = imgp.tile([128, IMG_CH, 512], F16, tag="wch", name="wch")
            eng = sync if c % 2 == 0 else act
            eng.dma_start(wch[:, :, 0:256], io["imgwt"][c][:, :, 0:256])
            eng2 = act if c % 2 == 0 else sync
            eng2.dma_start(wch[:, :, 256:512], io["imgwt"][c][:, :, 256:512])
            for k in range(IMG_CH):
                kk = c * IMG_CH + k
                for m in range(4):
                    pe.matmul(ps_img[m][:],
                              wch[:, k, m * 128:(m + 1) * 128],
                              enct[:, kk, :],
                              start=(kk == 0), stop=(kk == KT_IMG - 1))
        ximg_f = imgp.tile([128, 512], F16, name="ximg_f")
        for m in range(4):
            act.copy(ximg_f[:, m * 128:(m + 1) * 128], ps_img[m][:])
        ar_in = dramp.tile([128, 512], F16, name="ar_in")
        ar_out = dramp.tile([128, 512], F16, name="ar_out", addr_space="Shared")
        sync.dma_start(ar_in[:], ximg_f[:])
        gps.collective_compute("AllReduce", mybir.AluOpType.add,
                               replica_groups=RG,
                               ins=[ar_in.opt()], outs=[ar_out.opt()])
        # big recurrence weights stream in during the AllReduce wait
        sync.dma_start(whh[:, 0:4, :], io["whh_t"][:, 0:4, :])
        act.dma_start(whh[:, 4:8, :], io["whh_t"][:, 4:8, :])
        sync.dma_start(fcw[:, 0:4, :], io["fcw_t"][:, 0:4, :])
        act.dma_start(fcw[:, 4:8, :], io["fcw_t"][:, 4:8, :])
        sync.dma_start(xemb[:, 0:2, :, :], io["embt"][:, 0:2, :, :])
        act.dma_start(xemb[:, 2:4, :, :], io["embt"][:, 2:4, :, :])
        ximg_full = imgp.tile([128, 512], F16, name="ximg_full")
        sync.dma_start(ximg_full[:], ar_out[:])
        for m in range(4):
            dve.tensor_scalar_add(ximg[:, m, :],
                                  ximg_full[:, m * 128:(m + 1) * 128],
                                  imgb[:, m:m + 1])

    # ---- recurrence ----
    with tc.tile_pool(name="psg", bufs=1, space="PSUM") as psg, \
         tc.tile_pool(name="psf", bufs=1, space="PSUM") as psf, \
         tc.tile_pool(name="dyn", bufs=2) as dyn, \
         tc.tile_pool(name="agp", bufs=2, space="DRAM") as agp:

        hk_prev = None  # 8 SBUF tiles [128,128] f16: full h.T of prev step

        def emit_fc_mm(hk):
            """fc matmuls for a step; returns psum tiles."""
            tiles = []
            off = 0
            for n, nw in enumerate(FC_N):
                psn = psf.tile([128, 512], F32, tag=f"psf{n}", name=f"psf{n}")
                if has_fcb:
                    pe.matmul(psn[:, :nw], ones1[:], fcb[:, off:off + nw],
                              start=True, stop=False)
                for k in range(8):
                    pe.matmul(psn[:, :nw], hk[k],
                              fcw[:, k, off:off + nw],
                              start=(k == 0 and not has_fcb), stop=(k == 7))
                tiles.append(psn)
                off += nw
            return tiles

        def emit_fc_out(s, tiles):
            """mask + store preds for step s (emitted off the critical path)."""
            off = 0
            for n, nw in enumerate(FC_N):
                pr = dyn.tile([128, 512], F32, tag=f"pr{n}", name=f"pr{n}")
                act.activation(pr[:, :nw], tiles[n][:, :nw], AF.Copy,
                               scale=mask[:, s:s + 1])
                gps.dma_start(io["preds"][:, s, off:off + nw], pr[:, :nw])
                off += nw

        fc_tiles = None
        for t in range(T):
            # gates.T accumulation, one PSUM bank per gate m-tile.
            # x-side first (no AG dependency -> fills the AG wait window),
            # then the h-side after the AllGather of the previous step.
            ps_g = [psg.tile([128, 128], F32, tag=f"psg{m}", name=f"psg{m}")
                    for m in range(4)]
            for m in range(4):
                for k in range(4):
                    xsl = ximg[:, k, :] if t == 0 else xemb[:, k, t - 1, :]
                    pe.matmul(ps_g[m][:],
                              wih[:, k, m * 128:(m + 1) * 128], xsl,
                              start=(k == 0), stop=(t == 0 and k == 3))
            if t > 0:
                # k-outer: each k-block only needs hk[k], so the PE chases
                # the 8 h-tile DMA arrivals instead of waiting for the last
                for k in range(8):
                    for m in range(4):
                        pe.matmul(ps_g[m][:],
                                  whh[:, k, m * 128:(m + 1) * 128],
                                  hk_prev[k],
                                  start=False, stop=(k == 7))
                # fc of previous step rides the PE queue after this step's
                # gates (same AG dependency, off the critical chain)
                fc_tiles = emit_fc_mm(hk_prev)

            # activations in dependency-friendly order: i, g first (-> i*g),
            # then f (-> f*c), then o
            g_act = [None] * 4
            for m in (0, 2, 1, 3):
                a = dyn.tile([128, 128], F32, tag=f"ga{m}", name=f"ga{m}")
                act.activation(a[:], ps_g[m][:],
                               AF.Tanh if m == 2 else AF.Sigmoid,
                               bias=bias[:, m:m + 1])
                g_act[m] = a
            i_g, f_g, g_g, o_g = g_act
            if t == 0:
                dve.tensor_mul(c_st[:], i_g[:], g_g[:])
            else:
                t2 = dyn.tile([128, 128], F32, tag="t2", name="t2")
                t1 = dyn.tile([128, 128], F32, tag="t1", name="t1")
                dve.tensor_mul(t2[:], i_g[:], g_g[:])
                dve.tensor_mul(t1[:], f_g[:], c_st[:])
                dve.tensor_add(c_st[:], t1[:], t2[:])
            tc_ = dyn.tile([128, 128], F32, tag="tc", name="tc_")
            act.activation(tc_[:], c_st[:], AF.Tanh)
            h16 = dyn.tile([128, 128], F16, tag="h16", name="h16")
            dve.tensor_mul(h16[:], o_g[:], tc_[:])

            # exchange: shard -> full h.T
            bnc = agp.tile([128, 128], F16, tag="bnc", name="bnc")
            gat = agp.tile([HID, 128], F16, tag="gat", name="gat",
                           addr_space="Shared")
            act.dma_start(bnc[:], h16[:])
            gps.collective_compute("AllGather", mybir.AluOpType.bypass,
                                   replica_groups=RG,
                                   ins=[bnc.opt()], outs=[gat.opt()])
            hk = []
            for p in range(4):
                ht = dyn.tile([128, 2, 128], F16, tag=f"hkp{p}", name=f"hkp{p}")
                eng = sync if p % 2 == 0 else act
                eng.dma_start(ht[:], gat.rearrange("(k p) b -> p k b", p=128)
                              [:, 2 * p:2 * p + 2, :])
                hk.append(ht)
            hk_prev = [hk[k // 2][:, k % 2, :] for k in range(8)]
            # preds of the previous step: emitted last, off the critical path
            if t > 0:
                emit_fc_out(t - 1, fc_tiles)

        fc_tiles = emit_fc_mm(hk_prev)
        emit_fc_out(T - 1, fc_tiles)

    es.close()


def _build(has_fcb):
    nc = bacc.Bacc("TRN2", target_bir_lowering=False, debug=False,
                   num_devices=N_CORES)
    io = {
        "whh_t": nc.dram_tensor("whh_t", [128, 8, 512], F16, kind="ExternalInput").ap(),
        "wih_t": nc.dram_tensor("wih_t", [128, 4, 512], F16, kind="ExternalInput").ap(),
        "fcw_t": nc.dram_tensor("fcw_t", [128, 8, VSH], F16, kind="ExternalInput").ap(),
        "fcb": nc.dram_tensor("fcb", [1, VSH], F16, kind="ExternalInput").ap(),
        "bias": nc.dram_tensor("bias", [128, 4], F32, kind="ExternalInput").ap(),
        "imgb": nc.dram_tensor("imgb", [128, 4], F32, kind="ExternalInput").ap(),
        "mask": nc.dram_tensor("mask", [128, T], F32, kind="ExternalInput").ap(),
        "embt": nc.dram_tensor("embt", [128, 4, T - 1, 128], F16, kind="ExternalInput").ap(),
        "enct": nc.dram_tensor("enct", [128, KT_IMG, 128], F16, kind="ExternalInput").ap(),
        "imgwt": nc.dram_tensor("imgwt", [N_CH, 128, IMG_CH, 512], F16, kind="ExternalInput").ap(),
        "preds": nc.dram_tensor("preds", [128, T, VSH], F32, kind="ExternalOutput").ap(),
    }
    with tile.TileContext(nc) as tc:
        _emit(nc, tc, io, has_fcb)
    nc.compile()
    return nc


_PROGRAM = {}


def _program(has_fcb):
    if has_fcb not in _PROGRAM:
        _PROGRAM[has_fcb] = _build(has_fcb)
    return _PROGRAM[has_fcb]


def _host_prep(inputs):
    enc = np.asarray(inputs["encoder_out"], dtype=np.float32)
    caps = np.asarray(inputs["encoded_captions"])
    clen = np.asarray(inputs["caption_lengths"])
    emb_W = np.asarray(inputs["emb_W"], dtype=np.float32)
    img_W = np.asarray(inputs["img_W"], dtype=np.float32)
    img_b = np.asarray(inputs["img_b"], dtype=np.float32)
    W_ih = np.asarray(inputs["W_ih"], dtype=np.float32)
    W_hh = np.asarray(inputs["W_hh"], dtype=np.float32)
    b_ih = np.asarray(inputs["b_ih"], dtype=np.float32)
    b_hh = np.asarray(inputs["b_hh"], dtype=np.float32)
    fc_W = np.asarray(inputs["fc_W"], dtype=np.float32)
    fc_b = np.asarray(inputs["fc_b"], dtype=np.float32)

    lens = clen[:, 0]
    sort_ind = np.argsort(-lens, kind="stable")
    lens_s = lens[sort_ind]
    dec_len = lens_s - 1
    caps_s = caps[sort_ind]

    encf = enc.reshape(B, -1)[sort_ind]                      # [B, 100352]
    enct = np.ascontiguousarray(encf.T).astype(np.float16)   # [100352, B]
    imgwt = np.ascontiguousarray(img_W.T).astype(np.float16)  # [100352, 512]

    emb_x = emb_W[caps_s[:, :T - 1]]                         # [B, 18, EMB]
    # [EMB, 18, B] -> [128, 4, 18, 128] (partition-major SBUF layout)
    embt = np.ascontiguousarray(
        emb_x.transpose(2, 1, 0).reshape(4, 128, T - 1, 128)
        .transpose(1, 0, 2, 3))
    embt = embt.astype(np.float16)

    bsum = b_ih + b_hh
    maskf = (dec_len[:, None] >= np.arange(T)[None, :]).astype(np.float32)
    imgb_t = np.ascontiguousarray(img_b.reshape(4, 128).T).astype(np.float32)
    W_ihT = W_ih.T  # [EMB, 4H]
    W_hhT = W_hh.T  # [HID, 4H]
    fc_WT = fc_W.T  # [HID, VOCAB]

    def sbuf_km(a, kt):  # [kt*128, M] -> [128, kt, M]
        return np.ascontiguousarray(
            a.reshape(kt, 128, a.shape[1]).transpose(1, 0, 2))

    in_maps = []
    for j in range(N_CORES):
        rows = np.concatenate(
            [np.arange(g * HID + j * HSH, g * HID + (j + 1) * HSH)
             for g in range(4)])
        vsl = slice(j * VSH, (j + 1) * VSH)
        ksl = slice(j * KSH, (j + 1) * KSH)
        imgw_j = imgwt[ksl].reshape(N_CH, IMG_CH, 128, 512).transpose(0, 2, 1, 3)
        in_maps.append({
            "whh_t": sbuf_km(W_hhT[:, rows].astype(np.float16), 8),
            "wih_t": sbuf_km(W_ihT[:, rows].astype(np.float16), 4),
            "fcw_t": sbuf_km(fc_WT[:, vsl].astype(np.float16), 8),
            "fcb": np.ascontiguousarray(fc_b[vsl])[None, :].astype(np.float16),
            "bias": np.ascontiguousarray(
                bsum[rows].reshape(4, HSH).T).astype(np.float32),
            "imgb": imgb_t,
            "mask": maskf,
            "embt": embt,
            "enct": sbuf_km(enct[ksl], KT_IMG),
            "imgwt": np.ascontiguousarray(imgw_j),
        })
    meta = dict(sort_ind=sort_ind, dec_len=dec_len, caps_s=caps_s,
                has_fcb=bool(np.any(fc_b)))
    return in_maps, meta


def _install_trace_shim():
    """Make run_bass_kernel_spmd(trace=True) work under axon on this image."""
    import types
    try:
        import antenv  # noqa
        if "antenv.axon_hooks" not in sys.modules:
            mod = types.ModuleType("antenv.axon_hooks")
            _hook = [None]
            mod.set_axon_ntff_profile_hook = lambda h: _hook.__setitem__(0, h)
            mod.get_axon_ntff_profile_hook = lambda: _hook[0]
            sys.modules["antenv.axon_hooks"] = mod
            antenv.axon_hooks = mod
        from trn_agent_boot.trn_boot import _ntff_profile_via_ctypes
        sys.modules["antenv.axon_hooks"].set_axon_ntff_profile_hook(
            _ntff_profile_via_ctypes("/opt/axon/libaxon_pjrt.so"))
        import concourse.bass_utils as bu
        bu.upload_artifacts = lambda tmpdir: f"local:{tmpdir}"
    except Exception:
        pass


def kernel(**inputs):
    trace = bool(os.environ.get("BASS_TRACE"))
    if trace:
        _install_trace_shim()
    in_maps, meta = _host_prep(inputs)
    nc = _program(meta["has_fcb"])
    res = run_bass_kernel_spmd(nc, in_maps, list(range(N_CORES)), trace=trace)
    LAST_RESULT[0] = res
    preds = np.concatenate(
        [res.results[c]["preds"] for c in range(N_CORES)], axis=2)
    caps_out = meta["caps_s"]
    dec_out = meta["dec_len"]
    sort_out = meta["sort_ind"].astype(np.int32)
    return preds, caps_out, dec_out, sort_out
